# revision 2
# baseline (speedup 1.0000x reference)
"""Trainium2 Bass kernel for a pre-norm transformer block (attention + MLP).

Contract: kernel(**inputs) takes the FULL unsharded inputs of
nn_Block_33775622815825 and returns the FULL output. The batch (B=128) is
sharded data-parallel across 8 NeuronCores (16 per core); the whole block
runs per-core with no collectives.

v3: fp8 (e4m3) DoubleRow matmuls for QKV / MLP1 / MLP2 (measured 1.7x the
bf16 mm rate at these shapes; scores+attn+proj stay bf16), SBUF residuals
via fused DVE scalar_tensor_tensor, and a software-pipelined emission
order tuned against PE idle (idle >3us drops the PE clock ramp):
 - attention groups interleave into the score loop (attn grp g emitted
   after score block 2g+3's matmuls), V halves at blocks 0/2,
 - each pair's second proj is deferred past the NEXT pair's QK matmuls so
   the attnT xbar transpose lands while the PE chews QK,
 - the rowsum is a fused 65th column of V (v_sb[:, head, 64] = 1).
The scores keep the baseline's exact K=64-scores + K=128-mask-matmul
accumulation pattern: consecutive K=64 matmuls at different row offsets
without the interposed full-K matmul crash the runtime (PE erratum).

fp8 layout trick: LN emits h directly in fp8 [t, c]; the DMA-xbar
transpose (2-byte granules) transposes PAIRS of c-values, so
hT2[p, q, t, j] = h[t, 256q + 2p + j], and that pair index j is the
DoubleRow k-tile dim.  Weights are host-interleaved to match:
W8i[p, q, j, d] = W[256q + 2p + j, d].  V uses the same operands as
plain fp8 matmuls (j-sliced stationary; DoubleRow ldweights rejects the
1-byte-stride k-tile layout).

Scales (fp8e4 max-finite is 240): wq/wk/wv x128 (drain undoes), w1 x32
(hid carries x32, max ~110), w2 x128 (the fused residual multiply undoes
the x4096); measured rel err ~1.45e-2 vs fp32 reference (gate 2e-2).
"""
import os
import sys

import numpy as np

for _p in ("/opt/trn_rl_repo",):
    if _p not in sys.path:
        sys.path.insert(0, _p)

import ml_dtypes

import concourse.bass as bass
import concourse.tile as tile
from concourse import bacc
from concourse import mybir
from concourse.bass import ds, ts

F32 = mybir.dt.float32
BF16 = mybir.dt.bfloat16
FP8 = mybir.dt.float8e4
U16 = mybir.dt.uint16
AF = mybir.ActivationFunctionType
ALU = mybir.AluOpType
DR = mybir.MatmulPerfMode.DoubleRow

B, T, C, H, D = 128, 128, 1024, 16, 64
C4 = 4 * C
NCORES = 8
BPC = B // NCORES  # batch elements per core
EPS = 1e-5
SCL = float(C) ** -0.5  # softmax scale (1/32)
NEG = -1.0e9

SW_QKV = 128.0  # fp8 scale on wq/wk/wv
SW_1 = 32.0     # fp8 scale on w1 (hid carries it)
SW_2 = 128.0    # fp8 scale on w2
INV_QKV = 1.0 / SW_QKV
INV_MLP2 = 1.0 / (SW_1 * SW_2)

bf16 = ml_dtypes.bfloat16
e4m3 = ml_dtypes.float8_e4m3


class _Bacc(bacc.Bacc):
    """Bacc whose activation-table pass resolves every function we use to
    the single `natural_log_exp_and_others` set (exp+ln+relu+copy live
    there together), so the kernel pays exactly one ACT_TABLE_LOAD."""

    _ONE_SET = "natural_log_exp_and_others"
    _OURS = {AF.Exp, AF.Ln, AF.Copy, AF.Relu, AF.Identity, AF.Square}

    def insert_act_table_loads(self):
        import bass_rust as _br

        from concourse.hw_specs import get_activation_tables

        has_activation = any(
            isinstance(i, mybir.InstActivation)
            for b in self.main_func.blocks
            for i in b.instructions
        )
        if not has_activation:
            return
        tables = []
        for name, funcs in get_activation_tables(self.m.arch).items():
            if name != self._ONE_SET:
                funcs = set(funcs) - self._OURS
            tables.append((name, funcs))
        _br.insert_act_table_loads(self, tables)


def build_program(bpc=BPC, qkv_bias=False, v_bias=False):
    """Emit the Bass/Tile program for one core handling `bpc` batch elems."""
    assert bpc % 2 == 0
    nc = _Bacc()

    xin = nc.declare_dram_parameter("xin", [bpc, T, C], F32, isOutput=False)
    # fp8 qkv weights, c-pair interleaved: [c_part, {q,k}, cq, j, d]
    wqk8 = nc.declare_dram_parameter("wqk8", [128, 2, 4, 2, C], FP8, isOutput=False)
    wv8 = nc.declare_dram_parameter("wv8", [128, 4, 2, C], FP8, isOutput=False)
    wp = nc.declare_dram_parameter("wp", [128, 8, C], BF16, isOutput=False)
    w18 = nc.declare_dram_parameter("w18", [128, 4, 2, C4], FP8, isOutput=False)
    w28 = nc.declare_dram_parameter("w28", [128, 32, C], FP8, isOutput=False)
    maskm = nc.declare_dram_parameter("maskm", [128, 128], BF16, isOutput=False)
    ident = nc.declare_dram_parameter("ident", [128, 128], BF16, isOutput=False)
    onesr = nc.declare_dram_parameter("onesr", [1, 256], BF16, isOutput=False)
    rows = nc.declare_dram_parameter("rows", [1, 2, C], BF16, isOutput=False)
    r1c = nc.declare_dram_parameter("r1c", [128, 32], F32, isOutput=False)
    if qkv_bias:
        qkr = nc.declare_dram_parameter("qkr", [1, 2, 8, 128], BF16, isOutput=False)
    if v_bias:
        vr = nc.declare_dram_parameter("vr", [1, C], BF16, isOutput=False)
    yout = nc.declare_dram_parameter("yout", [bpc, T, C], F32, isOutput=True)
    x1d = nc.dram_tensor("x1d", [bpc, T, C], F32)

    xin, yout, x1d = xin[:], yout[:], x1d[:]
    wqk8_d, wv8_d, wp_d, w18_d, w28_d = wqk8[:], wv8[:], wp[:], w18[:], w28[:]
    maskm_d, ident_d = maskm[:], ident[:]
    onesr_d, rows_d, r1c_d = onesr[:], rows[:], r1c[:]

    npairs = bpc // 2

    with tile.TileContext(nc) as tc:
        from contextlib import ExitStack

        with ExitStack() as ctx:
            consts = ctx.enter_context(tc.tile_pool(name="consts", bufs=1))
            wqkpool = ctx.enter_context(tc.tile_pool(name="wqkpool", bufs=1))
            wvppool = ctx.enter_context(tc.tile_pool(name="wvppool", bufs=1))
            w1pool = ctx.enter_context(tc.tile_pool(name="w1pool", bufs=1))
            w2pool = ctx.enter_context(tc.tile_pool(name="w2pool", bufs=1))
            xpool = ctx.enter_context(tc.tile_pool(name="xpool", bufs=6))
            hpool = ctx.enter_context(tc.tile_pool(name="hpool", bufs=2))
            htpool = ctx.enter_context(tc.tile_pool(name="htpool", bufs=3))
            respool = ctx.enter_context(tc.tile_pool(name="respool", bufs=2))
            stats = ctx.enter_context(tc.tile_pool(name="stats", bufs=4))
            psacc = ctx.enter_context(
                tc.tile_pool(name="psacc", bufs=2, space="PSUM")
            )

            # constants
            maskm_sb = consts.tile([128, 128], BF16)
            nc.gpsimd.dma_start(out=maskm_sb, in_=maskm_d)
            ident_sb = consts.tile([128, 128], BF16)
            nc.gpsimd.dma_start(out=ident_sb, in_=ident_d)
            ones_sb = consts.tile([1, 256], BF16)
            nc.gpsimd.dma_start(out=ones_sb, in_=onesr_d)
            rows_sb = consts.tile([1, 2, C], BF16)
            nc.gpsimd.dma_start(out=rows_sb, in_=rows_d)
            r1_sb = consts.tile([128, 32], F32)
            nc.gpsimd.dma_start(out=r1_sb, in_=r1c_d)
            eps_sb = consts.tile([128, 1], F32)
            nc.vector.memset(eps_sb, EPS)
            if qkv_bias:
                qkr_sb = consts.tile([1, 2, 8, 128], BF16)
                nc.gpsimd.dma_start(out=qkr_sb, in_=qkr[:])
            if v_bias:
                vr_sb = consts.tile([1, C], BF16)
                nc.gpsimd.dma_start(out=vr_sb, in_=vr[:])

            wqk_sb = wqkpool.tile([128, 2, 4, 2, C], FP8)
            wv_sb = wvppool.tile([128, 4, 2, C], FP8, tag="wv")
            wp_sb = wvppool.tile([128, 8, C], BF16, tag="wp")
            w1_sb = w1pool.tile([128, 4, 2, C4], FP8)
            w2_sb = w2pool.tile([128, 32, C], FP8)

            def layernorm(x_sb, tagp):
                """LN stats on [128, C] fp32; returns fp8 normalized tile."""
                st6 = stats.tile([128, 2, 6], F32, tag="st6" + tagp)
                nc.vector.bn_stats(out=st6[:, 0], in_=x_sb[:, 0:512])
                nc.vector.bn_stats(out=st6[:, 1], in_=x_sb[:, 512:1024])
                mv = stats.tile([128, 2], F32, tag="mv" + tagp)
                nc.vector.bn_aggr(out=mv, in_=st6)
                # rstd = exp(-0.5*ln(var+eps)) : stays in the exp/ln table set
                lnv = stats.tile([128, 1], F32, tag="lnv" + tagp)
                nc.scalar.activation(out=lnv, in_=mv[:, 1:2], func=AF.Ln, bias=eps_sb)
                rstd = stats.tile([128, 1], F32, tag="rstd" + tagp)
                nc.scalar.activation(out=rstd, in_=lnv, func=AF.Exp, scale=-0.5)
                rstd2 = stats.tile([128, 1], F32, tag="rstd2" + tagp)
                nc.vector.tensor_copy(out=rstd2, in_=rstd)
                h_sb = hpool.tile([128, C], FP8, tag="h")
                nc.vector.tensor_scalar(
                    out=h_sb, in0=x_sb, scalar1=mv[:, 0:1], scalar2=rstd2,
                    op0=ALU.subtract, op1=ALU.mult,
                )
                return h_sb

            def load_ln_transpose(b, hT2, pi, src, live):
                x_sb = xpool.tile([128, C], F32, tag="x", name=f"x_{b}")
                nc.gpsimd.dma_start(out=x_sb, in_=src[b])
                h_sb = layernorm(x_sb, "a")
                # packed-pair xbar transpose: 2-byte granules carry c-pairs
                nc.sync.dma_start(
                    out=hT2[:, :, pi], in_=h_sb.bitcast(U16), transpose=True
                )
                live[b] = x_sb

            x_live = {}
            x1_live = {}

            def frontA(pair):
                hT2 = htpool.tile([128, 4, 2, 128], U16, tag="ht",
                                  name=f"hT_{pair}")
                for pi in range(2):
                    load_ln_transpose(2 * pair + pi, hT2, pi, xin, x_live)
                return hT2

            def frontB(pair):
                h2T2 = htpool.tile([128, 4, 2, 128], U16, tag="ht",
                                   name=f"h2T_{pair}")
                for pi in range(2):
                    load_ln_transpose(2 * pair + pi, h2T2, pi, x1d, x1_live)
                return h2T2

            # ---------------- phase A ----------------
            with ExitStack() as actx:
                psatt = actx.enter_context(
                    tc.tile_pool(name="psatt", bufs=2, space="PSUM")
                )
                pswide = actx.enter_context(
                    tc.tile_pool(name="pswide", bufs=3, space="PSUM")
                )
                qkpool = actx.enter_context(tc.tile_pool(name="qkpool", bufs=2))
                vpool = actx.enter_context(tc.tile_pool(name="vpool", bufs=2))
                apool = actx.enter_context(tc.tile_pool(name="apool", bufs=3))
                weipool = actx.enter_context(tc.tile_pool(name="weipool", bufs=4))

                # startup weight loads: qkv on the idle scalar HWDGE queue,
                # wq first so the first QK matmuls start sooner
                nc.scalar.dma_start(out=wqk_sb[:, 0], in_=wqk8_d[:, 0])
                nc.scalar.dma_start(out=wqk_sb[:, 1], in_=wqk8_d[:, 1])
                front_next = frontA(0)
                nc.gpsimd.dma_start(out=wv_sb, in_=wv8_d)
                nc.gpsimd.dma_start(out=wp_sb, in_=wp_d)

                # mlp weight slabs streamed across phase A, flushed at the end
                pending = [(w1_sb[:, q], w18_d[:, q]) for q in range(4)]
                pending += [(w2_sb[:, ts(mc, 8)], w28_d[:, ts(mc, 8)])
                            for mc in range(4)]

                deferred_proj = [None]  # (attnT, b) awaiting proj

                def qk_pass(hT8):
                    qt_sb = qkpool.tile([128, 2, 8, 128], BF16, tag="qt")
                    kt_sb = qkpool.tile([128, 2, 8, 128], BF16, tag="kt")
                    for wi, dst in ((0, qt_sb), (1, kt_sb)):
                        for blk in range(8):
                            ps = psacc.tile([128, 2, 128], F32, tag="ps")
                            if qkv_bias:
                                nc.tensor.matmul(
                                    ps, lhsT=qkr_sb[0:1, wi, blk],
                                    rhs=ones_sb,
                                    start=True, stop=False,
                                )
                            for cq in range(4):
                                rhs = hT8[:, cq].rearrange(
                                    "p b (t j) -> p j (b t)", j=2)
                                nc.tensor.matmul(
                                    ps,
                                    lhsT=wqk_sb[:, wi, cq, :, ts(blk, 128)],
                                    rhs=rhs,
                                    start=(cq == 0 and not qkv_bias),
                                    stop=(cq == 3),
                                    perf_mode=DR,
                                )
                            nc.scalar.activation(
                                out=dst[:, :, blk], in_=ps, func=AF.Copy,
                                scale=INV_QKV,
                            )
                    return qt_sb, kt_sb

                def proj_pass(attnT, b):
                    # half-bank psum tiles: banks free as soon as each
                    # half's fused residual STT drains them
                    x1_sb = respool.tile([128, C], F32, tag="res",
                                         name=f"res_{b}")
                    for half in range(2):
                        hs = ds(half * 512, 512)
                        prp = pswide.tile([128, 512], F32, tag="wide",
                                          name=f"prp_{b}_{half}")
                        nc.tensor.matmul(
                            prp, lhsT=ones_sb[0:1, 0:128],
                            rhs=rows_sb[0:1, 0, hs],
                            start=True, stop=False,
                        )
                        for hc in range(8):
                            nc.tensor.matmul(
                                prp,
                                lhsT=attnT[:, hc],
                                rhs=wp_sb[:, hc, hs],
                                start=False, stop=(hc == 7),
                            )
                        # x1 = x + attn_proj + b_proj, residual in SBUF
                        nc.vector.scalar_tensor_tensor(
                            out=x1_sb[:, hs], in0=prp, scalar=1.0,
                            in1=x_live[b][:, hs],
                            op0=ALU.mult, op1=ALU.add,
                        )
                    nc.gpsimd.dma_start(out=x1d[b], in_=x1_sb)
                    del x_live[b]

                def zip_pass(hT8, pair, pi):
                    """Scores + exp + V + attn for one elem, interleaved so
                    the PE never starves while ACT/DVE/DMA chains drain."""
                    b = 2 * pair + pi
                    v_sb = vpool.tile([128, 16, 65], BF16, tag="v",
                                      name=f"v_{b}")
                    nc.vector.memset(v_sb[:, :, 64], 1.0)  # fused rowsum col
                    attnT = apool.tile([128, 8, 128], BF16, tag="attnT",
                                       name=f"attnT_{b}")
                    weiTs = []

                    def v_half(half):
                        psv = psacc.tile([128, 512], F32, tag="ps",
                                         name=f"psv_{b}_{half}")
                        dc = ds(half * 512, 512)
                        if v_bias:
                            nc.tensor.matmul(
                                psv, lhsT=ones_sb[0:1, 0:128],
                                rhs=vr_sb[0:1, dc],
                                start=True, stop=False,
                            )
                        first = not v_bias
                        for cq in range(4):
                            hj = hT8[:, cq, pi].rearrange(
                                "p (t j) -> p j t", j=2)
                            for j in range(2):
                                nc.tensor.matmul(
                                    psv, lhsT=hj[:, j],
                                    rhs=wv_sb[:, cq, j, dc],
                                    start=first,
                                    stop=(cq == 3 and j == 1),
                                )
                                first = False
                        # drain into the 65-col layout [*, head, 0:64]
                        nc.scalar.activation(
                            out=v_sb[:, ds(half * 8, 8), 0:64], in_=psv,
                            func=AF.Copy, scale=INV_QKV,
                        )

                    def attn_grp(grp):
                        att_ps = psatt.tile([128, 4, 65], F32, tag="att",
                                            bufs=1, name=f"att_{b}_{grp}")
                        for j in range(4):
                            hh = 4 * grp + j
                            blk, sub = hh // 2, hh % 2
                            nc.tensor.matmul(
                                att_ps[:, j],
                                lhsT=weiTs[blk][:, sub],
                                rhs=v_sb[:, hh],
                                start=True, stop=True,
                            )
                        rr4 = stats.tile([128, 4], F32, tag="rr")
                        nc.vector.reciprocal(out=rr4, in_=att_ps[:, :, 64])
                        attn_bf = weipool.tile([128, 4, 64], BF16,
                                               tag="anorm", bufs=4)
                        nc.vector.tensor_mul(
                            out=attn_bf, in0=att_ps[:, :, 0:64],
                            in1=rr4.to_broadcast([128, 4, 64]),
                        )
                        nc.sync.dma_start(
                            out=attnT[:, ts(grp, 2)], in_=attn_bf,
                            transpose=True,
                        )

                    for blk in range(8):
                        sc2 = psatt.tile([128, 2, 128], F32, tag="ps",
                                         name=f"sc_{b}_{blk}")
                        for sub in range(2):
                            po = sub * 64
                            # the K=128 mask mm both adds the causal -1e9 and
                            # keeps the PE off the consecutive-K=64 erratum
                            nc.tensor.matmul(
                                sc2[:, sub],
                                lhsT=kt_sb[po:po + 64, pi, blk],
                                rhs=qt_sb[po:po + 64, pi, blk],
                                start=True, stop=False,
                            )
                            nc.tensor.matmul(
                                sc2[:, sub], lhsT=maskm_sb, rhs=ident_sb,
                                start=False, stop=True,
                            )
                        weiT = weipool.tile([128, 2, 128], BF16, tag="weiT",
                                            bufs=10)
                        nc.scalar.activation(
                            out=weiT, in_=sc2, func=AF.Exp, scale=SCL,
                        )
                        weiTs.append(weiT)
                        if blk == 0:
                            v_half(0)
                        elif blk == 2:
                            v_half(1)
                        elif blk in (3, 5, 7):
                            attn_grp((blk - 3) // 2)
                    attn_grp(3)
                    return attnT

                for pair in range(npairs):
                    hT2 = front_next
                    hT8 = hT2.bitcast(FP8)  # [128, 4, 2, 256]
                    nslab = len(pending) if pair == npairs - 1 else 1
                    for _ in range(nslab):
                        if pending:
                            dst, src = pending.pop(0)
                            nc.gpsimd.dma_start(out=dst, in_=src)

                    if pair + 1 < npairs:
                        front_next = frontA(pair + 1)
                    qt_sb, kt_sb = qk_pass(hT8)
                    attnT0 = zip_pass(hT8, pair, 0)
                    # previous pair's deferred proj: its attnT transpose has
                    # had the whole QK + zip(0) stream to land
                    if deferred_proj[0] is not None:
                        proj_pass(*deferred_proj[0])
                    attnT1 = zip_pass(hT8, pair, 1)
                    proj_pass(attnT0, 2 * pair)
                    deferred_proj[0] = (attnT1, 2 * pair + 1)

                # last deferred proj must be emitted before frontB(0):
                # the x1d write and read ride the same gpsimd queue
                proj_pass(*deferred_proj[0])
                frontB_first = frontB(0)

            # ---------------- phase B ----------------
            with tc.tile_pool(name="hidpool", bufs=2) as hidpool, \
                    tc.tile_pool(name="ps1pool", bufs=3,
                                 space="PSUM") as ps1pool, \
                    tc.tile_pool(name="pswideB", bufs=3,
                                 space="PSUM") as pswideB:

                front_next = frontB_first
                for pair in range(npairs):
                    h2T2 = front_next
                    h2T8 = h2T2.bitcast(FP8)
                    if pair + 1 < npairs:
                        front_next = frontB(pair + 1)
                    hid8 = hidpool.tile([128, 2, 32, 128], FP8, tag="hid")
                    for mb in range(32):
                        ps1 = ps1pool.tile([128, 2, 128], F32, tag="ps")
                        for cq in range(4):
                            rhs = h2T8[:, cq].rearrange(
                                "p b (t j) -> p j (b t)", j=2)
                            nc.tensor.matmul(
                                ps1, lhsT=w1_sb[:, cq, :, ts(mb, 128)],
                                rhs=rhs,
                                start=(cq == 0), stop=(cq == 3),
                                perf_mode=DR,
                            )
                        # relu drains alternate ACT / DVE to balance engines
                        if mb % 2 == 0:
                            nc.scalar.activation(
                                out=hid8[:, :, mb], in_=ps1, func=AF.Relu,
                                bias=r1_sb[:, mb:mb + 1],
                            )
                        else:
                            nc.vector.tensor_scalar(
                                out=hid8[:, :, mb], in0=ps1,
                                scalar1=r1_sb[:, mb:mb + 1], scalar2=0.0,
                                op0=ALU.add, op1=ALU.max,
                            )
                    for pi in range(2):
                        b = 2 * pair + pi
                        # two half-bank psum tiles per elem; each bank's
                        # group starts at bank granularity (512 fp32 cols)
                        halves = [
                            pswideB.tile([128, 512], F32, tag="wideB",
                                         name=f"ps2_{b}_{hf}")
                            for hf in range(2)
                        ]
                        for hf in range(2):
                            nc.tensor.matmul(
                                halves[hf], lhsT=ones_sb[0:1, 0:128],
                                rhs=rows_sb[0:1, 1, ds(hf * 512, 512)],
                                start=True, stop=False,
                            )
                        for jp in range(16):
                            for cg in range(4):
                                # stop only on the last mm touching each
                                # 2KB psum zero region
                                nc.tensor.matmul(
                                    halves[cg // 2][:, ds((cg % 2) * 256, 256)],
                                    lhsT=hid8[:, pi, 2 * jp:2 * jp + 2, :],
                                    rhs=w2_sb[:, 2 * jp:2 * jp + 2,
                                              ds(cg * 256, 256)],
                                    start=False,
                                    stop=(jp == 15 and cg % 2 == 1),
                                    perf_mode=DR,
                                )
                        o_sb = respool.tile([128, C], F32, tag="res",
                                            name=f"out_{b}")
                        for hf in range(2):
                            hs = ds(hf * 512, 512)
                            nc.vector.scalar_tensor_tensor(
                                out=o_sb[:, hs], in0=halves[hf],
                                scalar=INV_MLP2,
                                in1=x1_live[b][:, hs],
                                op0=ALU.mult, op1=ALU.add,
                            )
                        nc.gpsimd.dma_start(out=yout[b], in_=o_sb)
                        del x1_live[b]

    nc.compile()
    return nc


def _interleave(w, scale):
    """[C, N] -> [128, 4, 2, N] with W8i[p, q, j, :] = W[256q + 2p + j, :]."""
    n = w.shape[1]
    return np.ascontiguousarray(
        (w * scale).reshape(4, 128, 2, n).transpose(1, 0, 2, 3)
    ).astype(e4m3)


def prep_host(inputs):
    """Host-side weight packing / folding. Returns (shared in_map, flags)."""
    f32 = np.float32
    wq = np.asarray(inputs["wq"], f32)
    wk = np.asarray(inputs["wk"], f32)
    wv = np.asarray(inputs["wv"], f32)
    w_proj = np.asarray(inputs["w_proj"], f32)
    b_proj = np.asarray(inputs["b_proj"], f32)
    w1 = np.asarray(inputs["w1"], f32)
    b1 = np.asarray(inputs["b1"], f32)
    w2 = np.asarray(inputs["w2"], f32)
    b2 = np.asarray(inputs["b2"], f32)
    g1 = np.asarray(inputs["ln1_g"], f32)
    bt1 = np.asarray(inputs["ln1_b"], f32)
    g2 = np.asarray(inputs["ln2_g"], f32)
    bt2 = np.asarray(inputs["ln2_b"], f32)

    wq_f = wq.transpose(1, 0, 2).reshape(C, C)  # [c, h*d]
    wk_f = wk.transpose(1, 0, 2).reshape(C, C)
    wv_f = wv.transpose(1, 0, 2).reshape(C, C)

    # fold LN1 gamma into qkv weights; LN1 beta becomes rank-1 rows
    rq = bt1 @ wq_f
    rk = bt1 @ wk_f
    rv = bt1 @ wv_f
    qkv_bias = bool(np.abs(rq).max() > 0 or np.abs(rk).max() > 0)
    v_bias = bool(np.abs(rv).max() > 0)

    wq8 = _interleave(g1[:, None] * wq_f, SW_QKV)  # [128, 4, 2, C]
    wk8 = _interleave(g1[:, None] * wk_f, SW_QKV)
    wv8 = _interleave(g1[:, None] * wv_f, SW_QKV)
    wqk8 = np.ascontiguousarray(np.stack([wq8, wk8], axis=1))

    wp_p = np.ascontiguousarray(
        w_proj.reshape(8, 128, C).transpose(1, 0, 2)).astype(bf16)

    w18 = _interleave(g2[:, None] * w1, SW_1)  # [128, 4, 2, 4C]
    w28 = np.ascontiguousarray(
        (w2 * SW_2).reshape(32, 128, C).transpose(1, 0, 2)).astype(e4m3)

    r1 = (bt2 @ w1 + b1) * SW_1  # pre-relu bias row, on the w1 scale
    r1c = np.ascontiguousarray(r1.reshape(32, 128).T, dtype=f32)  # [128, 32]

    # proj bias at true scale; mlp2 bias on the hid*w2 psum scale
    rows = np.stack([b_proj, b2 * SW_1 * SW_2], 0)[None]  # [1, 2, C]

    maskm = np.triu(np.full((128, 128), NEG, f32), 1)  # -1e9 iff s > t
    ident = np.eye(128, dtype=f32)
    onesr = np.ones((1, 256), f32)

    shared = {
        "wqk8": wqk8,
        "wv8": wv8,
        "wp": wp_p,
        "w18": w18,
        "w28": w28,
        "maskm": maskm.astype(bf16),
        "ident": ident.astype(bf16),
        "onesr": onesr.astype(bf16),
        "rows": np.ascontiguousarray(rows).astype(bf16),
        "r1c": r1c,
    }
    if qkv_bias:
        qkr = np.stack([rq, rk], 0).reshape(2, 8, 128)[None] * SW_QKV
        shared["qkr"] = np.ascontiguousarray(qkr).astype(bf16)
    if v_bias:
        shared["vr"] = (rv[None] * SW_QKV).astype(bf16)
    return shared, qkv_bias, v_bias


_CACHE = {}


def _get_program(bpc, qkv_bias, v_bias):
    key = (bpc, qkv_bias, v_bias)
    if key not in _CACHE:
        _CACHE[key] = build_program(bpc, qkv_bias, v_bias)
    return _CACHE[key]


def run(inputs, trace=False):
    from concourse.bass_utils import run_bass_kernel_spmd

    x = np.asarray(inputs["x"], np.float32)
    shared, qkv_bias, v_bias = prep_host(inputs)
    nc = _get_program(BPC, qkv_bias, v_bias)
    in_maps = []
    for i in range(NCORES):
        m = dict(shared)
        m["xin"] = np.ascontiguousarray(x[i * BPC:(i + 1) * BPC])
        in_maps.append(m)
    res = run_bass_kernel_spmd(
        nc, in_maps, core_ids=list(range(NCORES)), trace=trace
    )
    out = np.concatenate(
        [np.asarray(res.results[i]["yout"], np.float32) for i in range(NCORES)], 0
    )
    return out, res


def kernel(**inputs):
    out, _ = run(inputs, trace=False)
    return out


if __name__ == "__main__":
    nc = build_program(int(sys.argv[1]) if len(sys.argv) > 1 else 2)
    print("build ok")
